# revision 30
# baseline (speedup 1.0000x reference)
"""Trainium2 kernel for nn_Attention_64235530879045.

Mathematical structure of the reference module:
  v[b,h,m,d] = spe_agg[b, h*D+d]  (broadcast over sequence m), and
  softmax rows sum to 1, so  attn @ v == v  exactly:
    out[b,h,n,d] = sum_m attn[b,h,n,m] * v[b,h,d] = v[b,h,d].
  Therefore the module output is
    y[b,n,:] = spe_agg[b] @ W_proj.T + b_proj      (independent of n, x, W_qkv)
  broadcast over the N=1024 sequence positions (verified: rel err ~4e-7 vs the
  f32 reference).

Device strategy (8 NeuronCores, no collectives needed):
  Tensor-parallel over output channels: core i owns columns [96*i, 96*(i+1)).
  Raw bacc (no TileContext) to keep semaphore/barrier machinery minimal.
  Per core:
    1. y1 = spe_agg @ W_proj[cols].T + b_proj[cols]   (8 x 96 in fp32 PSUM;
       K=768 in 6 chunks of 128; a 7th one-hot chunk adds the bias; inputs
       arrive as bf16 in two DMAs so the PE starts early; fp32 matmul would
       run the PE in two-pass LOW_HIGH mode at twice the cost)
    2. y1 rows -> bf16 in the free dim of partition 0 (DVE cast copy + a
       small SBUF->SBUF DMA), at a 128-column pitch per batch so every
       access pattern stays within one PSUM bank
    3. partition-broadcast: ones[1,128].T @ y1flat -> bc[p, b*128+j] = y1[b,j]
       (two bf16 matmuls of 4 batches each, each within one PSUM bank; the
       ones row is carried in the wptb input)
    4. fan-out copies materialize R=4 physical fp32 repeats per batch
       (osb[p, b, c, j], 1536-B contiguous runs), DVE batches 0-3, ACT 4-7
    5. 2 output DMAs (4 batches each, 1.5 MB, SP + ACT HWDGE rings) into a
       p-major DRAM layout [128, B, NB/R, R*CS]; source repeats each
       1536-B run NB/R times.
  Host-side: transpose the p-major shard + concat channels.
"""

import numpy as np
import ml_dtypes

import concourse.bass as bass
import concourse.mybir as mybir
from concourse import bacc
from concourse.bass_utils import run_bass_kernel_spmd

# bass_utils' axon trace path imports antenv.axon_hooks unconditionally when
# BASS_TRACE is set; this container's antenv stub lacks it. Provide the hook
# (real NTFF profiling when the boot module is available, else a graceful
# no-op) so tracing never crashes the kernel.
try:
    import antenv.axon_hooks  # noqa: F401
except ImportError:
    import sys as _sys
    import types as _types

    def _make_ntff_hook():
        try:
            from trn_agent_boot.trn_boot import _ntff_profile_via_ctypes
            return _ntff_profile_via_ctypes("/opt/axon/libaxon_pjrt.so")
        except Exception:
            return None

    _hook = _make_ntff_hook()
    _m = _types.ModuleType("antenv.axon_hooks")
    _m.get_axon_ntff_profile_hook = lambda: _hook
    _sys.modules["antenv.axon_hooks"] = _m

B, N, C = 8, 1024, 768
N_CORES = 8
CS = C // N_CORES          # 96 output channels per core
KC = C // 128              # 6 contraction chunks
KCB = KC + 1               # + bias chunk
NB = N // 128              # 8 row repeats per partition; row n = p*8 + rep
R = 4                      # physical repeats materialized in SBUF
KA = 3                     # chunks in first wpt DMA
WCOLS_A = KA * CS                       # wpt part A: chunks 0..2
WCOLS_B = (KCB - KA) * CS + 128         # part B: chunks 3..6 + ones row
PITCH = 128                # per-batch column pitch in flat/bc (bank-aligned)

F32 = mybir.dt.float32
BF16 = mybir.dt.bfloat16
USE_BF16 = True            # bf16 matmul inputs; fp32 would run the PE in
                           # two-pass LOW_HIGH mode at 2x cost
IN_DT = BF16
IN_NP = ml_dtypes.bfloat16

_CACHE = {}


def _build():
    nc = bacc.Bacc("TRN2", target_bir_lowering=False, debug=False,
                   num_devices=N_CORES)

    spe_d = nc.dram_tensor("spe", [128, KCB * B], IN_DT, kind="ExternalInput")
    wpta_d = nc.dram_tensor("wpta", [128, WCOLS_A], IN_DT, kind="ExternalInput")
    wptb_d = nc.dram_tensor("wptb", [128, WCOLS_B], IN_DT, kind="ExternalInput")
    out_d = nc.dram_tensor("out", [B, 128, NB, CS], BF16,
                           kind="ExternalOutput")

    with (
        nc.sbuf_tensor([128, KCB * B], IN_DT) as spe_sb,
        nc.sbuf_tensor([128, WCOLS_A], IN_DT) as wpta_sb,
        nc.sbuf_tensor([128, WCOLS_B], IN_DT) as wptb_sb,
        nc.sbuf_tensor([128, CS], IN_DT) as y1_sb,
        nc.sbuf_tensor([1, B, PITCH], IN_DT) as flat,
        nc.sbuf_tensor([128, B, R, CS], BF16) as osb,
        nc.psum_tensor([128, CS], F32) as y1_ps,
        nc.psum_tensor([128, B, PITCH], F32) as bc_ps,
        nc.semaphore("s_sp") as s_sp,      # spe arrival (SP ring)
        nc.semaphore("s_wb") as s_wb,      # wptb arrival (SP ring)
        nc.semaphore("s_wa") as s_wa,      # wpta arrival (ACT ring)
        nc.semaphore("s_pe") as s_pe,      # y1 done
        nc.semaphore("s_y1") as s_y1,      # y1 copied to SBUF
        nc.semaphore("s_fl") as s_fl,      # flat ready
        nc.semaphore("s_bc") as s_bc,      # bc halves done (2)
        nc.semaphore("s_cpd") as s_cpd,    # osb fan-out, DVE pairs (b01, b23)
        nc.semaphore("s_cpa") as s_cpa,    # osb fan-out, ACT pairs (b45, b67)
        nc.semaphore("s_out") as s_out,    # output DMAs done (2*16)
    ):
        ones = wptb_sb[0:1, (KCB - KA) * CS:(KCB - KA) * CS + 128]

        block_cm = nc.Block()
        block = block_cm.__enter__()

        @block.tensor
        def _(pe):
            pe.wait_ge(s_wa, 16)
            pe.wait_ge(s_sp, 16)
            for k in range(KA):
                nc.tensor.matmul(
                    y1_ps[:B, :], spe_sb[:, k * B:(k + 1) * B],
                    wpta_sb[:, k * CS:(k + 1) * CS],
                    start=(k == 0), stop=False,
                )
            pe.wait_ge(s_wb, 16)
            for k in range(KA, KCB):
                j = k - KA
                mmres = nc.tensor.matmul(
                    y1_ps[:B, :], spe_sb[:, k * B:(k + 1) * B],
                    wptb_sb[:, j * CS:(j + 1) * CS],
                    start=False, stop=(k == KCB - 1),
                )
            mmres.then_inc(s_pe, 1)
            pe.wait_ge(s_fl, 16)
            # each half covers 4 batches at 128-col pitch = one PSUM bank
            nc.tensor.matmul(bc_ps[:, 0:4, :CS], ones,
                             flat[0:1, 0:4, :CS],
                             start=True, stop=True).then_inc(s_bc, 1)
            nc.tensor.matmul(bc_ps[:, 4:8, :CS], ones,
                             flat[0:1, 4:8, :CS],
                             start=True, stop=True).then_inc(s_bc, 1)

        @block.vector
        def _(dve):
            dve.wait_ge(s_pe, 1)
            nc.vector.tensor_copy(y1_sb[:B, :], y1_ps[:B, :]).then_inc(s_y1, 1)
            dve.wait_ge(s_bc, 1)
            for pair in (0, 1):
                b0 = pair * 2
                for c in range(R):
                    cp = nc.vector.tensor_copy(osb[:, b0:b0 + 2, c, :],
                                               bc_ps[:, b0:b0 + 2, :CS])
                cp.then_inc(s_cpd, 1)

        @block.scalar
        def _(act):
            act.dma_start(out=wpta_sb[:], in_=wpta_d[:]).then_inc(s_wa, 16)
            act.wait_ge(s_bc, 2)
            for pair in (0, 1):
                b0 = 4 + pair * 2
                for c in range(R):
                    cp = nc.scalar.copy(osb[:, b0:b0 + 2, c, :],
                                        bc_ps[:, b0:b0 + 2, :CS])
                cp.then_inc(s_cpa, 1)
                act.wait_ge(s_cpa, pair + 1)
                for b in (b0, b0 + 1):
                    src = (osb[:, b]
                           .rearrange("p c j -> p (c j)")
                           .unsqueeze(1).broadcast_to([128, NB // R, R * CS]))
                    act.dma_start(out=out_d[b], in_=src).then_inc(s_out, 16)

        @block.sync
        def _(sp):
            sp.dma_start(out=spe_sb[:], in_=spe_d[:]).then_inc(s_sp, 16)
            sp.dma_start(out=wptb_sb[:], in_=wptb_d[:]).then_inc(s_wb, 16)
            sp.wait_ge(s_y1, 1)
            sp.dma_start(out=flat[0:1, :, :CS],
                         in_=y1_sb[:B, :]).then_inc(s_fl, 16)
            for pair in (0, 1):
                sp.wait_ge(s_cpd, pair + 1)
                for b in (pair * 2, pair * 2 + 1):
                    src = (osb[:, b]
                           .rearrange("p c j -> p (c j)")
                           .unsqueeze(1).broadcast_to([128, NB // R, R * CS]))
                    sp.dma_start(out=out_d[b], in_=src).then_inc(s_out, 16)
            sp.wait_ge(s_out, 128)

        # Block exit emits per-engine drains + an all-engine barrier; clear
        # the kernel sems after it so the NEFF can be re-executed.
        block_cm.__exit__(None, None, None)
        for s in (s_sp, s_wb, s_wa, s_pe, s_y1, s_fl, s_bc, s_cpd, s_cpa,
                  s_out):
            nc.sync.sem_clear(s)

    nc.compile()
    return nc


def _prep_inputs(spe_agg, W_proj, b_proj):
    # spe_host[p, k*B+b] = spe_agg[b, k*128+p] for k<KC; chunk KC is the
    # bias selector: partition 0 row = ones, rest 0.
    spe_host = np.zeros((128, KCB, B), dtype=IN_NP)
    spe_host[:, :KC, :] = np.ascontiguousarray(spe_agg.T).reshape(
        KC, 128, B).transpose(1, 0, 2).astype(IN_NP)
    spe_host[0, KC, :] = 1.0
    spe_host = spe_host.reshape(128, KCB * B)

    wpt_full = np.ascontiguousarray(W_proj.T)          # (C, C): [c, j]
    in_maps = []
    for i in range(N_CORES):
        j0 = i * CS
        w = (wpt_full[:, j0:j0 + CS].reshape(KC, 128, CS)
             .transpose(1, 0, 2))                       # (128, KC, CS)
        wa = np.ascontiguousarray(w[:, :KA].reshape(128, WCOLS_A)).astype(IN_NP)
        wb = np.zeros((128, WCOLS_B), dtype=IN_NP)
        wb[:, :(KC - KA) * CS] = w[:, KA:].reshape(
            128, (KC - KA) * CS).astype(IN_NP)
        wb[0, (KC - KA) * CS:(KCB - KA) * CS] = b_proj[j0:j0 + CS].astype(IN_NP)
        wb[0, (KCB - KA) * CS:] = 1.0                   # ones row
        in_maps.append({"spe": spe_host, "wpta": wa, "wptb": wb})
    return in_maps


def kernel(x, spe_agg, W_qkv, W_proj, b_proj):
    # x and W_qkv do not affect the output (see module analysis above).
    spe_agg = np.ascontiguousarray(spe_agg, dtype=np.float32)
    W_proj = np.ascontiguousarray(W_proj, dtype=np.float32)
    b_proj = np.ascontiguousarray(b_proj, dtype=np.float32)

    if "nc" not in _CACHE:
        _CACHE["nc"] = _build()
    nc = _CACHE["nc"]

    in_maps = _prep_inputs(spe_agg, W_proj, b_proj)
    res = run_bass_kernel_spmd(nc, in_maps, core_ids=list(range(N_CORES)))
    # per-core out: (B, 128, NB, CS) with row n = p*8 + i -> (B, N, CS).
    # Device writes bf16; the values are exactly bf16-representable (y1 is
    # rounded to bf16 before the broadcast), so the f32 upcast is lossless.
    shards = [np.asarray(res.results[i]["out"]).astype(np.float32)
              .reshape(B, N, CS) for i in range(N_CORES)]
    return np.concatenate(shards, axis=2)
